# revision 2
# baseline (speedup 1.0000x reference)
"""Fused cross-attention kernel for 8 Trainium2 NeuronCores (bf16).

Key identity (each head uses the FULL 256-dim embedding -- source quirk):
  scores_h = L (Wq_h Wk_h^T) X^T  = L M_h X^T
  out      = sum_h softmax(scores_h * s) (X (Wv_h Wu_h)) + bu
           = sum_h Phat_h U_h + bu,   U_h = X N_h
M_h, N_h ([256,256] per head) are precomputed on the host for free,
eliminating the K/Q/V projections and the final unify matmul:
per-core PE work drops from 3.22G to 2.68G MACs; bf16 inputs halve
DMA to ~2MB and make LDWEIGHTS 1 cycle/row so every matmul pair is
stream-bound (f32r weight loads run at fp32 rate, 4 cyc/row, which
would make the N=256 phases weight-load-bound).

Sharding: core c = 2*a + hg handles batch a, head-group hg (4 heads).
Host sums the two partial outputs per batch element and adds the bias.

Device schedule (per core):
  TT_h = M_h^T L^T     [f,y]   lhsT=MM-slice, rhs=LT      (8 mm/head, N=512)
  S^T  = X TT_h        [b,y]   lhsT=XT-slice, rhs=TT      (16 mm/head-chunk)
  P^T  = exp(S^T * s)          ACT reads f32 psum, writes bf16
  d^T[y] per y-tile: bf16 add-tree (DVE+GPSIMD, DVE tail) then tiny
    N=1 matmuls lhsT=acc rhs=ones -> psum; DVE reciprocal [128,4]
  U_h  = X N_h         [s,c]   lhsT=XT-slice, rhs=NN-slice (16 mm/head)
  O    = sum_h P^T_h.T U_h     lhsT=pt-slice, rhs=U (8 mm/(h,yt), N=256)
    accumulated per head in PSUM; normalization (1/d) folded into the
    PSUM->SBUF eviction as a per-partition scale (O's partition dim is
    y, so 1/d is a [128,1] scalar -- no broadcast needed);
    cross-head sum on DVE/GPSIMD into o_acc, DMA'd out per y-tile.

All softmax bookkeeping is off the PE critical path: O matmuls need
only P^T tiles; the den matmuls are woven several microseconds after
their operands are ready so they never head-of-line block the PE queue.
"""

import math
import os
import sys

import numpy as np
import ml_dtypes

sys.path.insert(0, "/opt/trn_rl_repo")

import concourse.bass as bass  # noqa: E402
import concourse.mybir as mybir  # noqa: E402
from concourse import bacc  # noqa: E402
from concourse.bass_utils import run_bass_kernel_spmd  # noqa: E402
from concourse.tile import TileContext  # noqa: E402

F32 = mybir.dt.float32
BF16 = mybir.dt.bfloat16
EXP = mybir.ActivationFunctionType.Exp
COPY = mybir.ActivationFunctionType.Copy

B, S, E = 4, 1024, 256          # batch, seq, embed
N_CORES = 8
HG = 4                           # heads per core
SCALE = 1.0 / math.sqrt(32.0)

P = 128
ET = E // P                      # 2 embed partition tiles
ST = S // P                      # 8 seq partition tiles
NCH = 512                        # score-chunk moving width
YT_PER_CH = NCH // P             # 4 y-tiles per chunk

_CACHE = {}

PT_BUFS = int(os.environ.get("K2_PT", "20"))
SC_BUFS = int(os.environ.get("K2_SC", "3"))
O_BUFS = int(os.environ.get("K2_O", "4"))
DEN_POS = int(os.environ.get("K2_DENPOS", "6"))


def _build():
    nc = bacc.Bacc(target_bir_lowering=False)

    XT = nc.dram_tensor("XT", [E, S], BF16, kind="ExternalInput")
    LT = nc.dram_tensor("LT", [E, S], BF16, kind="ExternalInput")
    MMd = nc.dram_tensor("MM", [E, HG * E], BF16, kind="ExternalInput")
    NNd = nc.dram_tensor("NN", [E, HG * E], BF16, kind="ExternalInput")
    O = nc.dram_tensor("O", [S, E], F32, kind="ExternalOutput")

    with TileContext(nc) as tc:
        with tc.tile_pool(name="persist", bufs=1) as pp, \
             tc.tile_pool(name="tts", bufs=4) as tp, \
             tc.tile_pool(name="us", bufs=16) as upool, \
             tc.tile_pool(name="pts", bufs=PT_BUFS) as ptp, \
             tc.tile_pool(name="small", bufs=2) as mp, \
             tc.tile_pool(name="psum", bufs=1, space="PSUM") as ps:

            xt = [pp.tile([P, S], BF16, tag=f"xt{e}", name=f"xt{e}")
                  for e in range(ET)]
            lt = [pp.tile([P, S], BF16, tag=f"lt{e}", name=f"lt{e}")
                  for e in range(ET)]
            mmt = [pp.tile([P, HG * E], BF16, tag=f"mm{e}", name=f"mm{e}")
                   for e in range(ET)]
            nnt = [pp.tile([P, HG * E], BF16, tag=f"nn{e}", name=f"nn{e}")
                   for e in range(ET)]
            ones = nc.const_aps.aps[(BF16, 1.0)]  # [128,1], memset at init
            o_acc = [pp.tile([P, E], F32, tag=f"oa{yt}", name=f"oa{yt}")
                     for yt in range(ST)]

            # ---- input DMA: fill-critical stream on the sync queue
            # (gpsimd/scalar queues start later); XT on scalar, NN gpsimd ----
            for e in range(ET):
                nc.sync.dma_start(out=mmt[e][:, 0:E],
                                  in_=MMd[e * P:(e + 1) * P, 0:E])
            for e in range(ET):
                nc.sync.dma_start(out=lt[e][:, 0:NCH],
                                  in_=LT[e * P:(e + 1) * P, 0:NCH])
            for e in range(ET):
                nc.scalar.dma_start(out=xt[e][:],
                                    in_=XT[e * P:(e + 1) * P, :])
            for e in range(ET):
                nc.sync.dma_start(out=lt[e][:, NCH:S],
                                  in_=LT[e * P:(e + 1) * P, NCH:S])
            for e in range(ET):
                nc.sync.dma_start(out=mmt[e][:, E:HG * E],
                                  in_=MMd[e * P:(e + 1) * P, E:HG * E])
            for e in range(ET):
                nc.gpsimd.dma_start(out=nnt[e][:],
                                    in_=NNd[e * P:(e + 1) * P, :])

            # persistent per-head state
            tt = {}       # (h, fh) -> [128, S] bf16
            u = {}        # (h, st) -> [128, E] bf16
            pt = {}       # (h, bt) -> [128, S] bf16
            acc = {}      # (h, c) -> [128, NCH] bf16
            acc2 = {}
            rec = {}      # h -> [128, ST] f32 reciprocal denominators

            evict_ctr = [0]

            # gpsimd cannot access PSUM; evictions alternate DVE/ACT
            def evict(dst, src):
                evict_ctr[0] += 1
                if evict_ctr[0] % 2 == 0:
                    nc.vector.tensor_copy(dst, src)
                else:
                    nc.scalar.activation(dst, src, COPY)

            def tt_chunk(h, c):
                sl = bass.ts(c, NCH)
                for fh in range(2):
                    if (h, fh) not in tt:
                        tt[(h, fh)] = tp.tile([P, S], BF16, tag="tt",
                                              name=f"tt{h}{fh}")
                    pv = ps.tile([P, NCH], F32, tag="sc", bufs=SC_BUFS,
                                 name=f"ptt{h}{fh}{c}")
                    for e in range(ET):
                        nc.tensor.matmul(
                            pv[:],
                            mmt[e][:, h * E + fh * P: h * E + (fh + 1) * P],
                            lt[e][:, sl], start=(e == 0), stop=(e == ET - 1))
                    evict(tt[(h, fh)][:, sl], pv[:])

            def u_group(h, st_):
                if (h, st_) not in u:
                    u[(h, st_)] = upool.tile([P, E], BF16, tag="u",
                                             name=f"u{h}{st_}")
                pu = ps.tile([P, E], F32, tag="o", bufs=O_BUFS,
                             name=f"pu{h}{st_}")
                for e in range(ET):
                    nc.tensor.matmul(pu[:], xt[e][:, st_ * P:(st_ + 1) * P],
                                     nnt[e][:, h * E:(h + 1) * E],
                                     start=(e == 0), stop=(e == ET - 1))
                evict(u[(h, st_)][:], pu[:])

            def score_group(h, c, bt):
                sl = bass.ts(c, NCH)
                if (h, bt) not in pt:
                    pt[(h, bt)] = ptp.tile([P, S], BF16, tag="pt",
                                           name=f"pt{h}{bt}")
                pss = ps.tile([P, NCH], F32, tag="sc", bufs=SC_BUFS,
                              name=f"pss{h}{c}{bt}")
                for ft in range(2):
                    nc.tensor.matmul(pss[:], xt[ft][:, bt * P:(bt + 1) * P],
                                     tt[(h, ft)][:, sl],
                                     start=(ft == 0), stop=(ft == 1))
                nc.scalar.activation(pt[(h, bt)][:, sl], pss[:], EXP,
                                     scale=SCALE)
                # denominator add-tree: gpsimd does the middle, DVE the
                # tail so the post-exp serial chain is short
                a1, a2 = acc[(h, c)], acc2[(h, c)]
                p = lambda i: pt[(h, i)][:, sl]  # noqa: E731
                if bt == 2:
                    nc.vector.tensor_add(a1[:], p(0), p(1))
                elif bt == 4:
                    nc.gpsimd.tensor_add(a2[:], p(2), p(3))
                elif bt == 6:
                    nc.gpsimd.tensor_add(a2[:], a2[:], p(4))
                    nc.gpsimd.tensor_add(a2[:], a2[:], p(5))
                elif bt == 7:
                    nc.vector.tensor_add(a1[:], a1[:], p(6))

            def scores(h, c, dh=None, dc=None):
                # chunk-c score stream; den(dh, dc) woven at DEN_POS
                acc[(h, c)] = mp.tile([P, NCH], BF16, tag="acc",
                                      name=f"acc{h}{c}")
                acc2[(h, c)] = mp.tile([P, NCH], BF16, tag="acc2",
                                       name=f"acc2{h}{c}")
                for bt in range(ST):
                    score_group(h, c, bt)
                    if dh is not None and bt == DEN_POS:
                        den(dh, dc)

            def den(h, c):
                sl = bass.ts(c, NCH)
                a1 = acc[(h, c)]
                nc.vector.tensor_add(a1[:], a1[:], pt[(h, 7)][:, sl])
                nc.vector.tensor_add(a1[:], a1[:], acc2[(h, c)][:])
                if h not in rec:
                    rec[h] = mp.tile([P, ST], F32, tag="rec", name=f"rec{h}")
                dp = ps.tile([P, YT_PER_CH], F32, tag="d", bufs=1,
                             name=f"dp{h}{c}")
                for i in range(YT_PER_CH):
                    nc.tensor.matmul(dp[:, i:i + 1],
                                     a1[:, i * P:(i + 1) * P], ones,
                                     start=True, stop=True)
                nc.vector.reciprocal(
                    rec[h][:, c * YT_PER_CH:(c + 1) * YT_PER_CH], dp[:])

            add_ctr = [0]

            def o_group(h, yt):
                po = ps.tile([P, E], F32, tag="o", bufs=O_BUFS,
                             name=f"po{h}{yt}")
                for bt in range(ST):
                    nc.tensor.matmul(po[:],
                                     pt[(h, bt)][:, yt * P:(yt + 1) * P],
                                     u[(h, bt)][:],
                                     start=(bt == 0), stop=(bt == ST - 1))
                rsl = rec[h][:, yt:yt + 1]
                dst = o_acc[yt] if h == 0 else mp.tile(
                    [P, E], F32, tag="oh", bufs=4, name=f"oh{h}{yt}")
                if yt % 2 == 0:
                    nc.scalar.activation(dst[:], po[:], COPY, scale=rsl)
                else:
                    nc.vector.tensor_scalar_mul(dst[:], po[:], rsl)
                if h > 0:
                    eng = nc.vector if add_ctr[0] % 2 == 0 else nc.gpsimd
                    add_ctr[0] += 1
                    eng.tensor_add(o_acc[yt][:], o_acc[yt][:], dst[:])
                if h == HG - 1:
                    deng = nc.sync if yt % 2 == 0 else nc.scalar
                    deng.dma_start(out=O[yt * P:(yt + 1) * P, :],
                                   in_=o_acc[yt][:])

            # ---------------- schedule ----------------
            tt_chunk(0, 0)
            scores(0, 0)
            tt_chunk(0, 1)
            tt_chunk(1, 0)
            tt_chunk(1, 1)
            for st_ in range(ST):
                u_group(0, st_)
            scores(0, 1, 0, 0)
            for yt in range(YT_PER_CH):
                o_group(0, yt)
            for st_ in range(ST):
                u_group(1, st_)
                if st_ == 4:
                    den(0, 1)
            tt_chunk(2, 0)
            tt_chunk(2, 1)
            for yt in range(YT_PER_CH, ST):
                o_group(0, yt)

            for h in (1, 2):
                scores(h, 0)
                scores(h, 1, h, 0)
                for yt in range(YT_PER_CH):
                    o_group(h, yt)
                for st_ in range(ST):
                    u_group(h + 1, st_)
                    if st_ == 4:
                        den(h, 1)
                if h == 1:
                    tt_chunk(3, 0)
                    tt_chunk(3, 1)
                if h == 1:
                    for yt in range(YT_PER_CH, ST):
                        o_group(h, yt)

            scores(3, 0)
            scores(3, 1, 3, 0)
            # head-2's chunk-1 O groups fill the exp(3,*) drain window
            for yt in range(YT_PER_CH, ST):
                o_group(2, yt)
            for yt in range(YT_PER_CH):
                o_group(3, yt)
            den(3, 1)
            for yt in range(YT_PER_CH, ST):
                o_group(3, yt)

    nc.compile()
    return nc


def _host_mm_nn(Wk, Wq, Wv, Wu, hg):
    cols = slice(hg * HG * E, (hg + 1) * HG * E)
    Wq4 = Wq[:, cols].reshape(E, HG, E)
    Wk4 = Wk[:, cols].reshape(E, HG, E)
    Wv4 = Wv[:, cols].reshape(E, HG, E)
    Wu4 = Wu[cols, :].reshape(HG, E, E)
    MM = np.einsum('ehc,fhc->ehf', Wq4, Wk4, optimize=True).reshape(E, HG * E)
    NN = np.einsum('ehc,hco->eho', Wv4, Wu4, optimize=True).reshape(E, HG * E)
    bf = ml_dtypes.bfloat16
    return MM.astype(bf), NN.astype(bf)


def kernel(batch, latent, Wk, Wq, Wv, Wu, bu):
    batch = np.asarray(batch, dtype=np.float32)
    latent = np.asarray(latent, dtype=np.float32)
    Wk = np.asarray(Wk, dtype=np.float32)
    Wq = np.asarray(Wq, dtype=np.float32)
    Wv = np.asarray(Wv, dtype=np.float32)
    Wu = np.asarray(Wu, dtype=np.float32)
    bu = np.asarray(bu, dtype=np.float32)

    if "nc" not in _CACHE:
        _CACHE["nc"] = _build()
    nc = _CACHE["nc"]

    mmnn = [_host_mm_nn(Wk, Wq, Wv, Wu, hg) for hg in range(2)]
    bf = ml_dtypes.bfloat16

    in_maps = []
    for core in range(N_CORES):
        a, hg = core // 2, core % 2
        MM, NN = mmnn[hg]
        in_maps.append({
            "XT": np.ascontiguousarray(batch[a].T.astype(bf)),
            "LT": np.ascontiguousarray(latent[a].T.astype(bf)),
            "MM": MM,
            "NN": NN,
        })

    _CACHE["in_maps"] = in_maps
    res = run_bass_kernel_spmd(nc, in_maps, core_ids=list(range(N_CORES)))

    out = np.empty((B, S, E), dtype=np.float32)
    for a in range(B):
        out[a] = res.results[2 * a]["O"] + res.results[2 * a + 1]["O"] + bu
    return out
